# revision 20
# baseline (speedup 1.0000x reference)
"""LocalAttention Bass kernel for Trainium2 (8 NeuronCores).

Problem: B=4 H=8 T=8192 D=64, window=128, look_backward=1, causal.
Sharding: pure (B*H) data parallelism — 32 heads / 8 cores = 4 heads each,
processed as 2 head-pairs so q/k DMAs use all 128 SBUF partitions.

Device algorithm (per head, per 128-token window w):
  S^T[k, q] = K_w' @ Q_w^T      (keys on partitions, so the softmax
                                 reduction over keys can ride the PV matmul)
  P = exp(S^T * D^-0.5) * causal01
  [O^T; r] = [V | 1]^T @ P      (ones column baked into V gives row-sums)
Host divides O^T by r and transposes back.

v3 performance notes (fp32 baseline 305 us; v2 bf16 144 us):
  - all matmul operands bf16 (fp32 matmul = 4 cyc/col vs 1 for bf16)
  - PE warm-up burst at t=0: the HAM clock gate keeps an idle PE at
    1.2 GHz; ~7 us of dummy matmuls flips it to 2.4 GHz before the
    first real matmul (v2 ran 110 us of its 144 cold)
  - PV matmuls merged: the two windows sharing a V slot are computed by
    ONE N=256 matmul (pairblock P columns are contiguous); accumulation
    relies on PSUM has_written semantics (start=True on the group's
    first matmul clears the whole bank; later start=False matmuls
    overwrite untouched elements, accumulate touched ones) ->
    5 matmuls + 5 LDWEIGHTS per group-head instead of 8+8
  - software pipelining: group g+1's S matmuls are emitted BEFORE group
    g's PV so the in-order tensor queue never stalls the scalar engine;
    h0's sp is double-buffered (h1 hides behind h0's activation)
  - exp writes P directly as bf16; causal mask 1/3 vector + 2/3 gpsimd;
    output copy downcasts to bf16 (halves HBM writes)
  - CHUNK_W=16 with loads prefetched 2 groups early cuts the startup
    serial DMA from ~15 us to ~6 us

PSUM budget (8 banks): sp0 x2 bufs (4) + sp1 x1 (2) + op x2 (2).

Host-side shard prep (inside kernel(), numpy, all bf16):
  qTp [2, 128, 8320]  — head-pair Q^T (head A rows 0-63, head B rows
                        64-127), one zero window appended (lookahead pad)
  kT  [2, 128, 8192]  — head-pair K^T
  vp  [4, 128, 4225]  — per head: 65 V slots x [V(64) | 1], slot 0 zeroed
  mask01 [128, 128]   — within-window causal 0/1 (k <= q)
Output:
  outT [4, 65, 8192] bf16 — rows 0..63 unnormalized O^T, row 64 row-sums
"""

import numpy as np

B, H, T, D = 4, 8, 8192, 64
W = 128                     # window size
WIN = T // W                # 64 windows per head
NCORES = 8
BH = B * H                  # 32
BH_PER_CORE = BH // NCORES  # 4
NPAIR = BH_PER_CORE // 2    # 2 head pairs per core
G = 4                       # windows per softmax group (PSUM tile = [128, 1024])
# chunk sizes in windows: a tiny first chunk gets the pipeline started
# ~7us earlier; the rest amortize DMA descriptor overhead
CHUNK_SIZES = [4, 12, 16, 16, 16]
assert sum(CHUNK_SIZES) == WIN
SCALE = float(D) ** -0.5

N_WARM = 20                 # PE warm-up matmuls (K=128, N=512 each)

_nc_cache = {}
last_perf = None


def _build_nc(skip=()):
    import concourse.tile as tile
    from concourse import bacc
    from concourse import mybir
    from contextlib import ExitStack

    f32 = mybir.dt.float32
    bf16 = mybir.dt.bfloat16
    Exp = mybir.ActivationFunctionType.Exp
    mult = mybir.AluOpType.mult

    nc = bacc.Bacc()
    qTp = nc.dram_tensor("qTp", [NPAIR, W, (WIN + 1) * W], bf16,
                         kind="ExternalInput")
    kT = nc.dram_tensor("kT", [NPAIR, W, T], bf16, kind="ExternalInput")
    vp = nc.dram_tensor("vp", [BH_PER_CORE, W, (WIN + 1) * (D + 1)], bf16,
                        kind="ExternalInput")
    mask = nc.dram_tensor("mask01", [W, W], bf16, kind="ExternalInput")
    outT = nc.dram_tensor("outT", [BH_PER_CORE, D + 1, T], bf16,
                          kind="ExternalOutput")

    with tile.TileContext(nc) as tc, ExitStack() as ctx:
        cpool = ctx.enter_context(tc.tile_pool(name="cpool", bufs=1))
        qpool = ctx.enter_context(tc.tile_pool(name="qpool", bufs=2))
        kpool = ctx.enter_context(tc.tile_pool(name="kpool", bufs=2))
        vpool = ctx.enter_context(tc.tile_pool(name="vpool", bufs=2))
        opool = ctx.enter_context(tc.tile_pool(name="opool", bufs=3))
        ppool = ctx.enter_context(tc.tile_pool(name="ppool", bufs=4))
        spsum0 = ctx.enter_context(tc.tile_pool(name="spsum0", bufs=2,
                                                space="PSUM"))
        spsum1 = ctx.enter_context(tc.tile_pool(name="spsum1", bufs=1,
                                                space="PSUM"))
        opsum = ctx.enter_context(tc.tile_pool(name="opsum", bufs=2,
                                               space="PSUM"))

        mtile = cpool.tile([W, W], bf16)
        nc.sync.dma_start(mtile[:], mask[:])
        z128 = cpool.tile([W, W], bf16)      # P for the all-masked pad window
        nc.vector.memset(z128[:], 0.0)

        mm = nc.tensor.matmul

        # --- PE warm-up: flip the HAM clock gate to 2.4 GHz while the ---
        # --- first input DMAs are in flight (results are discarded).  ---
        # --- K=128 stationary so the FULL array lights up: half-array ---
        # --- (K=64) activity does not trip the HAM busy detector.     ---
        if "warm" not in skip and N_WARM:
            wz = cpool.tile([W, 4 * W], bf16)
            nc.vector.memset(wz[:], 0.0)
            wps = spsum0.tile([W, G * 2 * W], f32, tag="sp0")
            for _ in range(N_WARM):
                mm(wps[:, 0:4 * W], z128[:, :], wz[:],
                   start=True, stop=True)

        # chunk descriptors: (start_window, n_windows)
        CH = []
        w = 0
        for nw in CHUNK_SIZES:
            CH.append((w, nw))
            w += nw

        # flattened (chunk, group) schedule with S one group ahead of PV
        def s_phase(qc, kc, g, sps):
            """S^T pairblock matmuls for one group: h0 block then h1."""
            w0 = g * G
            for h in range(2):
                hb = h * 64
                for i in range(G):
                    wl = w0 + i
                    mm(sps[h][:, i * 256:(i + 1) * 256],
                       kc[hb:hb + 64, wl * W:(wl + 1) * W],
                       qc[hb:hb + 64, wl * W:(wl + 2) * W],
                       start=True, stop=True)

        gidx = 0

        def consume_phase(p, ck, g, gp, sps, pt_prev, ocs):
            """exp + mask + merged PV + output copy for one group.

            g: group index within the chunk; gp: within the head-pair.
            """
            nonlocal gidx
            w0 = g * G
            for h in range(2):
                vc = ck["vcs"][h]
                pt = ppool.tile([W, G * 2 * W], bf16, tag=f"pt{h}",
                                name=f"pt{h}")
                if "exp" not in skip:
                    nc.scalar.activation(pt[:], sps[h][:], Exp, scale=SCALE)

                # causal mask on T1 blocks (cols 0,256,512,768)
                pt3 = pt[:].rearrange("p (g x) -> p g x", x=2 * W)
                t1 = pt3[:, :, 0:W]
                mb = mtile[:, None, :].to_broadcast([W, G, W])
                if "mask" not in skip:
                    if gidx % 3 == 0:
                        nc.vector.tensor_tensor(t1, t1, mb, mult)
                    else:
                        nc.gpsimd.tensor_tensor(t1, t1, mb, mult)

                # merged PV + row-sums: 5 matmuls instead of 8.
                # start=True on the boundary matmul clears the whole op
                # bank's has_written bits; the N=256 matmuls then overwrite
                # untouched columns and accumulate touched ones.
                op = opsum.tile([D + 1, G * W], f32, tag="op", name="op")
                if "pv" not in skip:
                    if pt_prev[h] is not None:
                        t0src = pt_prev[h][:, G * 256 - W:G * 256]
                    else:
                        t0src = z128[:]
                    mm(op[:, 0:W],
                       vc[:, w0 * (D + 1):(w0 + 1) * (D + 1)],
                       t0src, start=True, stop=False)
                    for j in range(1, G):
                        mm(op[:, (j - 1) * W:(j + 1) * W],
                           vc[:, (w0 + j) * (D + 1):(w0 + j + 1) * (D + 1)],
                           pt[:, (j - 1) * 256:j * 256],
                           start=False, stop=False)
                    mm(op[:, (G - 1) * W:G * W],
                       vc[:, (w0 + G) * (D + 1):(w0 + G + 1) * (D + 1)],
                       pt[:, (G - 1) * 256:(G - 1) * 256 + W],
                       start=False, stop=True)

                # per-2-group output staging tiles -> earlier, smaller stores
                half = (gp % 2) * G * W
                if gp % 2 == 0:
                    ocs[h] = opool.tile([D + 1, 2 * G * W], bf16,
                                        tag="oc", name="oc")
                if "ocopy" not in skip:
                    nc.vector.tensor_copy(
                        ocs[h][:, half:half + G * W], op[:])
                pt_prev[h] = pt
                gidx += 1

        def store_phase(p, gp, ocs):
            """store the [65, 2*G*W] staging tiles after an odd pair-group."""
            c0 = (gp - 1) * G * W
            for h in range(2):
                nc.sync.dma_start(
                    outT[2 * p + h, :, c0:c0 + 2 * G * W],
                    ocs[h][:])
                ocs[h] = None

        def load_chunk(p, c):
            ws, nw = CH[c]
            c0 = ws * W
            qc = qpool.tile([W, (nw + 1) * W], bf16, tag=f"qc{nw}",
                            name="qc")
            kc = kpool.tile([W, nw * W], bf16, tag=f"kc{nw}", name="kc")
            if "loads" not in skip:
                nc.sync.dma_start(qc[:], qTp[p, :, c0:c0 + (nw + 1) * W])
                nc.sync.dma_start(kc[:], kT[p, :, c0:c0 + nw * W])
            vcs = []
            for h in range(2):
                vc = vpool.tile([W, (nw + 1) * (D + 1)], bf16,
                                tag=f"vc{h}_{nw}", name=f"vc{h}")
                if "loads" not in skip:
                    v0 = ws * (D + 1)
                    nc.sync.dma_start(
                        vc[:],
                        vp[2 * p + h, :, v0:v0 + (nw + 1) * (D + 1)])
                vcs.append(vc)
            return dict(qc=qc, kc=kc, vcs=vcs, c=c, p=p)

        # global schedule
        sched = [(p, c, g) for p in range(NPAIR) for c in range(len(CH))
                 for g in range(CH[c][1] // G)]
        n = len(sched)
        gpp = n // NPAIR     # groups per pair
        chunks = {}          # (p, c) -> chunk tiles
        pt_prev_by_p = {p: [None, None] for p in range(NPAIR)}
        ocs_by_p = {p: [None, None] for p in range(NPAIR)}
        sp_of = {}           # i -> sps tiles for sched[i]

        def ensure_chunk(i):
            if i >= n:
                return
            p, c, g = sched[i]
            if (p, c) not in chunks:
                chunks[(p, c)] = load_chunk(p, c)

        ensure_chunk(0)

        def emit_s(i):
            p, c, g = sched[i]
            ensure_chunk(i)
            ck = chunks[(p, c)]
            sp0 = spsum0.tile([W, G * 2 * W], f32, tag="sp0", name="sp0")
            sp1 = spsum1.tile([W, G * 2 * W], f32, tag="sp1", name="sp1")
            sps = [sp0, sp1]
            sp_of[i] = sps
            if "smm" not in skip:
                s_phase(ck["qc"], ck["kc"], g, sps)

        emit_s(0)
        for i in range(n):
            p, c, g = sched[i]
            # prefetch the chunk needed 2 groups ahead
            ensure_chunk(i + 2)
            # S matmuls for the NEXT group go in front of this group's PV
            if i + 1 < n:
                emit_s(i + 1)
            ck = chunks[(p, c)]
            gp = i - p * gpp
            consume_phase(p, ck, g, gp, sp_of.pop(i), pt_prev_by_p[p],
                          ocs_by_p[p])
            if gp % 2 == 1 and "store" not in skip:
                store_phase(p, gp, ocs_by_p[p])
    nc.finalize()
    return nc


def _prep_core_inputs(q2, k2, v2, core):
    import ml_dtypes
    bf16 = ml_dtypes.bfloat16
    s0 = core * BH_PER_CORE
    qTp = np.zeros((NPAIR, W, (WIN + 1) * W), bf16)
    kTp = np.zeros((NPAIR, W, T), bf16)
    for p in range(NPAIR):
        for h in range(2):
            bh = s0 + 2 * p + h
            qTp[p, h * 64:(h + 1) * 64, :T] = q2[bh].T.astype(bf16)
            kTp[p, h * 64:(h + 1) * 64, :] = k2[bh].T.astype(bf16)
    vr = v2[s0:s0 + BH_PER_CORE].reshape(
        BH_PER_CORE, WIN, W, D).transpose(0, 2, 1, 3)
    vp = np.zeros((BH_PER_CORE, W, WIN + 1, D + 1), bf16)
    vp[:, :, 1:, :D] = vr.astype(bf16)
    vp[:, :, :, D] = 1.0
    vp = np.ascontiguousarray(vp.reshape(BH_PER_CORE, W, (WIN + 1) * (D + 1)))
    mask01 = (np.arange(W)[:, None] <= np.arange(W)[None, :]).astype(bf16)
    return {"qTp": qTp, "kT": kTp, "vp": vp, "mask01": mask01}


def kernel(q, k, v, _trace=False):
    global last_perf
    from concourse.bass_utils import run_bass_kernel_spmd

    q = np.ascontiguousarray(np.asarray(q), dtype=np.float32)
    k = np.ascontiguousarray(np.asarray(k), dtype=np.float32)
    v = np.ascontiguousarray(np.asarray(v), dtype=np.float32)
    q2 = q.reshape(BH, T, D)
    k2 = k.reshape(BH, T, D)
    v2 = v.reshape(BH, T, D)

    if "nc" not in _nc_cache:
        _nc_cache["nc"] = _build_nc()
    nc = _nc_cache["nc"]

    in_maps = [_prep_core_inputs(q2, k2, v2, core) for core in range(NCORES)]
    res = run_bass_kernel_spmd(
        nc, in_maps, core_ids=list(range(NCORES)), trace=_trace)
    last_perf = res

    outs = []
    for core in range(NCORES):
        ot = np.asarray(res.results[core]["outT"], dtype=np.float32)
        o = ot[:, :D, :] / ot[:, D:D + 1, :]           # normalize
        outs.append(o.transpose(0, 2, 1))              # [4, T, 64]
    full = np.concatenate(outs, axis=0)                # [32, T, 64]
    return full.reshape(B, H, T, D)
